# revision 29
# baseline (speedup 1.0000x reference)
"""Trainium2 Bass kernel for a 2-layer spiking LSTM (SLSTM) + FC readout.

Contract: kernel(**inputs) takes the FULL unsharded inputs and returns
the FULL [256, 8] output. Internally the batch is sharded 32-per-core
across 8 NeuronCores (data parallel, weights replicated); the T=400
time scan runs fully SBUF-resident per core.

v2 design (software-pipelined, layer 2 lags layer 1 by one iteration):

  - Gates for one layer in one PSUM bank [128, 512] f32:
      partition p = 32*jc + b, free n = 128*gt + hp, gt in [i, f, o, g].
    4-way PE column tiling streams the (bf16) weights; the state slice
    [128, 32] is stationary.
  - One 512-wide Sigmoid ACT per layer: the g-gate weight rows are
    pre-scaled x2 on the host so tanh(g) = 2*sigmoid(2g) - 1; the
    2s-1 folds into fused scalar_tensor_tensor DVE ops:
      p1 = (sg2 - 0.5) * si ;  syn' = 2*p1 + (sf * syn)
  - The PE transposes ht = so*tanh(syn') (pre-reset); the reset
    subtraction happens post-transpose, fused into the PSUM->SBUF
    cast:  mT = htT - thr*spkT_prev  (spike == reset indicator).
  - spkT = (mT > thr).  No separate reset tile, no scalar copy.
  - Layer 2 is emitted one iteration behind layer 1; its elementwise
    chain overlaps layer-1's matmuls.  A sync=False (order-only)
    dependency keeps layer-1's critical m1T/sp1 ahead of layer-2's q1
    in the DVE FIFO — this is worth ~850ns/step; sync=True semaphore
    edges or paced keepalive matmuls both destabilize the schedule and
    make the HAM clock-gate oscillation worse (measured).
    PSUM pools: g1 x2, g2 x3, tp x2, junk x1 banks.
"""

import sys

sys.path.insert(0, "/opt/trn_rl_repo")

import numpy as np
import ml_dtypes

T, B, I, H, C = 400, 256, 14, 512, 8
N_CORES = 8
BL = B // N_CORES  # 32
GATE_PERM = [0, 1, 3, 2]  # PyTorch gate rows [i,f,g,o] -> our order [i,f,o,g]

_cache = {}


def _scale_g(W: np.ndarray, f: float) -> np.ndarray:
    """Scale the g-gate block (PyTorch row order i,f,g,o)."""
    W = np.array(W, np.float32, copy=True)
    W[2 * H:3 * H] *= f
    return W


def _reorder_w(W: np.ndarray) -> np.ndarray:
    """[2048, Kin] (rows i,f,g,o) -> [128, KC*2048] bf16 streaming layout.

    free index = kc*2048 + jc*512 + gt*128 + hp, partition = k (h within
    contraction chunk kc)."""
    Kin = W.shape[1]
    KC = Kin // 128
    Wg = W.reshape(4, 4, 128, KC, 128)[GATE_PERM]  # [gt, jc, hp, kc, k]
    return np.ascontiguousarray(
        Wg.transpose(4, 3, 1, 0, 2).reshape(128, KC * 2048)
    ).astype(ml_dtypes.bfloat16)


def _reorder_w_small(Waug: np.ndarray) -> np.ndarray:
    """[2048, Kin<=128] -> [Kin, 2048] bf16; free = jc*512 + gt*128 + hp."""
    Kin = Waug.shape[1]
    Wg = Waug.reshape(4, 4, 128, Kin)[GATE_PERM]  # [gt, jc, hp, k]
    return np.ascontiguousarray(
        Wg.transpose(3, 1, 0, 2).reshape(Kin, 2048)
    ).astype(ml_dtypes.bfloat16)


def _reorder_b(b: np.ndarray) -> np.ndarray:
    bg = b.reshape(4, 4, 128)[GATE_PERM]  # [gt, jc, hp]
    return np.ascontiguousarray(
        bg.transpose(1, 0, 2).reshape(1, 2048)
    ).astype(ml_dtypes.bfloat16)


def build_nc(thr1: float, thr2: float, t_steps: int):
    import concourse.bacc as bacc
    import concourse.mybir as mybir
    from concourse import tile, masks
    from concourse.tile import add_dep_helper

    f32 = mybir.dt.float32
    bf16 = mybir.dt.bfloat16
    AF = mybir.ActivationFunctionType
    OP = mybir.AluOpType

    nc = bacc.Bacc("TRN2", target_bir_lowering=False, debug=False,
                   num_devices=N_CORES)

    d_x = nc.dram_tensor("xin", [15, t_steps * BL], bf16, kind="ExternalInput")
    d_wih1 = nc.dram_tensor("wih1", [15, 2048], bf16, kind="ExternalInput")
    d_whh1 = nc.dram_tensor("whh1", [128, 4 * 2048], bf16,
                            kind="ExternalInput")
    d_w2 = nc.dram_tensor("w2", [128, 8 * 2048], bf16, kind="ExternalInput")
    d_b2 = nc.dram_tensor("b2r", [1, 2048], bf16, kind="ExternalInput")
    d_out = nc.dram_tensor("msum", [128, 128], f32, kind="ExternalOutput")

    with tile.TileContext(nc) as tc:
        with (
            tc.tile_pool(name="const", bufs=1) as cpool,
            tc.tile_pool(name="state", bufs=1) as spool,
            tc.tile_pool(name="gs", bufs=2) as gspool,
            tc.tile_pool(name="tmp", bufs=2) as tpool,
            tc.tile_pool(name="tsb", bufs=2) as tsbpool,
            tc.tile_pool(name="g1", bufs=2, space="PSUM") as g1pool,
            tc.tile_pool(name="g2", bufs=3, space="PSUM") as g2pool,
            tc.tile_pool(name="tp", bufs=2, space="PSUM") as tppool,
            tc.tile_pool(name="jk", bufs=1, space="PSUM") as jkpool,
        ):
            x_sb = cpool.tile([15, t_steps * BL], bf16, tag="x")
            wih1 = cpool.tile([15, 2048], bf16, tag="wih1")
            whh1 = cpool.tile([128, 4 * 2048], bf16, tag="whh1")
            w2 = cpool.tile([128, 8 * 2048], bf16, tag="w2")
            b2r = cpool.tile([1, 2048], bf16, tag="b2r")
            ident = cpool.tile([128, 128], bf16, tag="ident")
            ones = cpool.tile([1, BL], bf16, tag="ones")

            nc.sync.dma_start(x_sb[:], d_x[:])
            nc.sync.dma_start(wih1[:], d_wih1[:])
            nc.sync.dma_start(whh1[:], d_whh1[:])
            nc.sync.dma_start(w2[:], d_w2[:])
            nc.sync.dma_start(b2r[:], d_b2[:])
            masks.make_identity(nc, ident[:])
            nc.gpsimd.memset(ones[:], 1.0)

            syn1 = spool.tile([128, 128], f32, tag="syn1")
            syn2 = spool.tile([128, 128], f32, tag="syn2")
            msum = spool.tile([128, 128], f32, tag="msum")
            m1T = spool.tile([128, 128], bf16, tag="m1T0")
            m2T = spool.tile([128, 128], bf16, tag="m2T0")
            sp1 = spool.tile([128, 128], bf16, tag="sp10")
            sp2 = spool.tile([128, 128], bf16, tag="sp20")
            for s in (syn1, syn2, msum):
                nc.vector.memset(s[:], 0.0)
            for s in (m1T, m2T, sp1, sp2):
                nc.gpsimd.memset(s[:], 0.0)

            def mm(psum, lhs, rhs, jc, start, stop):
                return nc.tensor.matmul(
                    psum[32 * jc:32 * jc + 32, :], lhs, rhs,
                    start=start, stop=stop, tile_position=(0, 32 * jc),
                    skip_group_check=True)

            def x_round(g1t, t):
                xsl = x_sb[:, t * BL:(t + 1) * BL]
                return [mm(g1t, xsl, wih1[:, 512 * jc:512 * jc + 512], jc,
                           True, False) for jc in range(4)]

            def bias_round(g2t):
                return [mm(g2t, ones[0:1, :],
                           b2r[0:1, 512 * jc:512 * jc + 512], jc,
                           True, False) for jc in range(4)]

            def whh1_rounds(g1t, lhsT):
                for kc in range(4):
                    lhs = lhsT[:, 32 * kc:32 * kc + 32]
                    for jc in range(4):
                        off = 2048 * kc + 512 * jc
                        mm(g1t, lhs, whh1[:, off:off + 512], jc,
                           False, kc == 3)

            def wih2_rounds(g2t, lhsT):
                for kc in range(4):  # w2 chunks 0..3 = W_ih2
                    lhs = lhsT[:, 32 * kc:32 * kc + 32]
                    for jc in range(4):
                        off = 2048 * kc + 512 * jc
                        mm(g2t, lhs, w2[:, off:off + 512], jc, False, False)

            def whh2_rounds(g2t, lhsT):
                for kc in range(4):  # w2 chunks 4..7 = W_hh2, closes group
                    lhs = lhsT[:, 32 * kc:32 * kc + 32]
                    for jc in range(4):
                        off = 2048 * (4 + kc) + 512 * jc
                        mm(g2t, lhs, w2[:, off:off + 512], jc,
                           False, kc == 3)

            junk = jkpool.tile([128, 512], f32, tag="junk")

            def junk_mm(anchor=None):
                j = nc.tensor.matmul(
                    junk[0:32, :], ones[0:1, :], b2r[0:1, 0:512],
                    start=True, stop=True, tile_position=(0, 0),
                    skip_group_check=True)
                if anchor is not None:
                    add_dep_helper(j.ins, anchor.ins, sync=True,
                                   reason="ham keepalive")
                return j

            # ---- prologue ----
            # HAM warm-up: ~4us of junk matmuls so the PE clock ungates
            for _ in range(12):
                junk_mm()

            g1_cur = g1pool.tile([128, 512], f32, tag="g1")
            x_round(g1_cur, 0)
            whh1_rounds(g1_cur, m1T)  # m1T == 0: closes g1(0) uniformly
            g2t0 = g2pool.tile([128, 512], f32, tag="g2")
            g2_open = {0: g2t0}
            bias_round(g2t0)

            l2 = None  # staged layer-2 work: (g2 tile, sp1 tile, m2T_prev)

            for k in range(t_steps + 1):
                do1 = k < t_steps  # layer-1 step k exists
                s = k - 1          # layer-2 step handled this iteration

                # 1. close g2(s): wih2 @ spk1(s) + whh2 @ m2T(s-1)
                if l2 is not None:
                    g2s, sp1s, m2Tprev = l2
                    wih2_rounds(g2s, sp1s)
                    whh2_rounds(g2s, m2Tprev)

                # 2. open g1(k+1) / g2(k+1)
                g1_next = None
                if do1 and k + 1 < t_steps:
                    g1_next = g1pool.tile([128, 512], f32, tag="g1")
                    x_round(g1_next, k + 1)
                if do1 and k + 1 <= t_steps - 1:
                    g2n = g2pool.tile([128, 512], f32, tag="g2")
                    bias_round(g2n)
                    g2_open[k + 1] = g2n

                # 3. layer-1 elementwise chain + transpose + whh1(k+1)
                if do1:
                    gs1 = gspool.tile([128, 512], f32, tag="gs1")
                    nc.scalar.activation(gs1[:], g1_cur[:], AF.Sigmoid)
                    si, sf = gs1[:, 0:128], gs1[:, 128:256]
                    so, sg2 = gs1[:, 256:384], gs1[:, 384:512]
                    p1 = tpool.tile([128, 128], bf16, tag="p1")
                    i_p1 = nc.vector.scalar_tensor_tensor(
                        p1[:], sg2, 0.5, si, OP.subtract, OP.mult)
                    # p2 on DVE right after p1: gpsimd picked it up ~1us
                    # late (FIFO quirks), stalling syn1 on the L1 cycle
                    p2 = tpool.tile([128, 128], bf16, tag="p2")
                    i_p2 = nc.vector.tensor_tensor(p2[:], sf, syn1[:],
                                                   OP.mult)
                    i_sy1 = nc.vector.scalar_tensor_tensor(
                        syn1[:], p1[:], 2.0, p2[:], OP.mult, OP.add)
                    tc1 = tpool.tile([128, 128], f32, tag="tc1")
                    i_tc1 = nc.scalar.activation(tc1[:], syn1[:], AF.Tanh)
                    ht1 = tsbpool.tile([128, 128], bf16, tag="ht1")
                    nc.vector.tensor_tensor(ht1[:], so, tc1[:], OP.mult)

                    tp1 = tppool.tile([128, 128], bf16, tag="tp")
                    nc.tensor.transpose(tp1[:], ht1[:], ident[:])
                    m1T_new = tsbpool.tile([128, 128], bf16, tag="m1T")
                    sp1_new = tsbpool.tile([128, 128], bf16, tag="sp1")
                    with tc.high_priority():
                        nc.vector.scalar_tensor_tensor(
                            m1T_new[:], sp1[:], -thr1, tp1[:],
                            OP.mult, OP.add)
                        i_sp1 = nc.vector.tensor_scalar(
                            sp1_new[:], m1T_new[:], thr1, None, OP.is_gt)

                    if g1_next is not None:
                        whh1_rounds(g1_next, m1T_new)
                else:
                    i_sp1 = i_p2 = None

                # 4. layer-2 elementwise chain for step s (lagged)
                if s >= 0:
                    g2s = l2[0]
                    gs2 = gspool.tile([128, 512], f32, tag="gs2")
                    nc.scalar.activation(gs2[:], g2s[:], AF.Sigmoid)
                    si2, sf2 = gs2[:, 0:128], gs2[:, 128:256]
                    so2, sg22 = gs2[:, 256:384], gs2[:, 384:512]
                    q2 = tpool.tile([128, 128], f32, tag="q2")
                    i_q2 = nc.gpsimd.tensor_tensor(q2[:], sf2, syn2[:],
                                                   OP.mult)
                    q1 = tpool.tile([128, 128], f32, tag="q1")
                    i_q1 = nc.vector.scalar_tensor_tensor(
                        q1[:], sg22, 0.5, si2, OP.subtract, OP.mult)
                    if i_sp1 is not None:
                        # order-only edge (no HW semaphore): keep layer-1's
                        # critical m1T/sp1 ahead of q1 in the DVE FIFO
                        add_dep_helper(i_q1.ins, i_sp1.ins, sync=False,
                                       reason="dve fifo order")
                    i_sy2 = nc.vector.scalar_tensor_tensor(
                        syn2[:], q1[:], 2.0, q2[:], OP.mult, OP.add)
                    tc2t = tpool.tile([128, 128], f32, tag="tc2")
                    i_tc2 = nc.scalar.activation(tc2t[:], syn2[:], AF.Tanh)
                    ht2 = tsbpool.tile([128, 128], bf16, tag="ht2")
                    nc.vector.tensor_tensor(ht2[:], so2, tc2t[:], OP.mult)

                    tp2 = tppool.tile([128, 128], bf16, tag="tp")
                    nc.tensor.transpose(tp2[:], ht2[:], ident[:])
                    m2T_new = tsbpool.tile([128, 128], bf16, tag="m2T")
                    i_m2T = nc.vector.scalar_tensor_tensor(
                        m2T_new[:], sp2[:], -thr2, tp2[:], OP.mult, OP.add)
                    if i_p2 is not None:
                        # order-only: keep layer-2's m2T behind layer-1's
                        # p1/p2 in the DVE FIFO (p1 measured +508ns queue
                        # delay behind m2T/sp2 at the sigma1 boundary)
                        add_dep_helper(i_m2T.ins, i_p2.ins, sync=False,
                                       reason="dve order p2 before m2T")
                    sp2_new = tsbpool.tile([128, 128], bf16, tag="sp2")
                    nc.vector.tensor_scalar(sp2_new[:], m2T_new[:], thr2,
                                            None, OP.is_gt)
                    nc.gpsimd.tensor_tensor(msum[:], msum[:], m2T_new[:],
                                            OP.add)
                    m2T, sp2 = m2T_new, sp2_new

                # 5. stage next iteration's layer-2 work
                if do1:
                    l2 = (g2_open.pop(k), sp1_new, m2T)
                    m1T, sp1 = m1T_new, sp1_new
                    if g1_next is not None:
                        g1_cur = g1_next
                else:
                    l2 = None

            nc.sync.dma_start(d_out[:], msum[:])

    nc.compile()
    return nc


def prep_core_inputs(x, W_ih1, W_hh1, b_ih1, b_hh1, W_ih2, W_hh2,
                     b_ih2, b_hh2, t_steps):
    """Shared (weight) arrays + per-core x shards.  The g-gate rows are
    pre-scaled x2 so the kernel can use tanh(g) = 2*sigmoid(2g) - 1."""
    b1 = _scale_g(np.asarray(b_ih1, np.float32)
                  + np.asarray(b_hh1, np.float32), 2.0)
    wih1_aug = np.concatenate(
        [_scale_g(np.asarray(W_ih1, np.float32), 2.0), b1[:, None]],
        axis=1)  # [2048, 15]
    wih1_r = _reorder_w_small(wih1_aug)  # [15, 2048]
    whh1_r = _reorder_w(_scale_g(np.asarray(W_hh1, np.float32), 2.0))
    w2cat = np.concatenate(
        [_scale_g(np.asarray(W_ih2, np.float32), 2.0),
         _scale_g(np.asarray(W_hh2, np.float32), 2.0)],
        axis=1)  # [2048, 1024]
    w2_r = _reorder_w(w2cat)  # [128, 16384]
    b2_r = _reorder_b(_scale_g(np.asarray(b_ih2, np.float32)
                               + np.asarray(b_hh2, np.float32), 2.0))

    x = np.asarray(x, np.float32)[:t_steps]
    in_maps = []
    for c in range(N_CORES):
        xs = x[:, c * BL:(c + 1) * BL, :]  # [T, 32, 14]
        xt = np.ascontiguousarray(
            xs.transpose(2, 0, 1).reshape(I, t_steps * BL))
        x_aug = np.concatenate(
            [xt, np.ones((1, t_steps * BL), np.float32)],
            axis=0).astype(ml_dtypes.bfloat16)  # [15, T*32]
        in_maps.append({
            "xin": x_aug,
            "wih1": wih1_r,
            "whh1": whh1_r,
            "w2": w2_r,
            "b2r": b2_r,
        })
    return in_maps


def unpack_msum(msum: np.ndarray, t_steps: int) -> np.ndarray:
    """[128, 128] transposed accumulator [hp, 32*jc+b] -> [32, 512]."""
    return (msum.reshape(128, 4, 32).transpose(2, 1, 0).reshape(32, 512)
            / np.float32(t_steps))


def kernel(x, W_ih1, W_hh1, b_ih1, b_hh1, thr1,
           W_ih2, W_hh2, b_ih2, b_hh2, thr2, W_fc, b_fc):
    from concourse.bass_utils import run_bass_kernel_spmd

    t_steps = x.shape[0]
    key = (float(thr1), float(thr2), t_steps)
    if key not in _cache:
        _cache[key] = build_nc(float(thr1), float(thr2), t_steps)
    nc = _cache[key]

    in_maps = prep_core_inputs(x, W_ih1, W_hh1, b_ih1, b_hh1,
                               W_ih2, W_hh2, b_ih2, b_hh2, t_steps)
    res = run_bass_kernel_spmd(nc, in_maps, list(range(N_CORES)))

    W_fc = np.asarray(W_fc, np.float32)
    b_fc = np.asarray(b_fc, np.float32)
    out = np.empty((B, C), np.float32)
    for c in range(N_CORES):
        mean_c = unpack_msum(res.results[c]["msum"], t_steps)  # [32, 512]
        out[c * BL:(c + 1) * BL] = mean_c @ W_fc.T + b_fc
    return out


# revision 30
# speedup vs baseline: 1.0040x; 1.0040x over previous
"""Trainium2 Bass kernel for a 2-layer spiking LSTM (SLSTM) + FC readout.

Contract: kernel(**inputs) takes the FULL unsharded inputs and returns
the FULL [256, 8] output. Internally the batch is sharded 32-per-core
across 8 NeuronCores (data parallel, weights replicated); the T=400
time scan runs fully SBUF-resident per core.

v2 design (software-pipelined, layer 2 lags layer 1 by one iteration):

  - Gates for one layer in one PSUM bank [128, 512] f32:
      partition p = 32*jc + b, free n = 128*gt + hp, gt in [i, f, o, g].
    4-way PE column tiling streams the (bf16) weights; the state slice
    [128, 32] is stationary.
  - One 512-wide Sigmoid ACT per layer: the g-gate weight rows are
    pre-scaled x2 on the host so tanh(g) = 2*sigmoid(2g) - 1; the
    2s-1 folds into fused scalar_tensor_tensor DVE ops:
      p1 = (sg2 - 0.5) * si ;  syn' = 2*p1 + (sf * syn)
  - The PE transposes ht = so*tanh(syn') (pre-reset); the reset
    subtraction happens post-transpose, fused into the PSUM->SBUF
    cast:  mT = htT - thr*spkT_prev  (spike == reset indicator).
  - spkT = (mT > thr).  No separate reset tile, no scalar copy.
  - Layer 2 is emitted one iteration behind layer 1; its elementwise
    chain overlaps layer-1's matmuls.  A sync=False (order-only)
    dependency keeps layer-1's critical m1T/sp1 ahead of layer-2's q1
    in the DVE FIFO — this is worth ~850ns/step; sync=True semaphore
    edges or paced keepalive matmuls both destabilize the schedule and
    make the HAM clock-gate oscillation worse (measured).
    PSUM pools: g1 x2, g2 x3, tp x2, junk x1 banks.
"""

import sys

sys.path.insert(0, "/opt/trn_rl_repo")

import numpy as np
import ml_dtypes

T, B, I, H, C = 400, 256, 14, 512, 8
N_CORES = 8
BL = B // N_CORES  # 32
GATE_PERM = [0, 1, 3, 2]  # PyTorch gate rows [i,f,g,o] -> our order [i,f,o,g]

_cache = {}


def _scale_g(W: np.ndarray, f: float) -> np.ndarray:
    """Scale the g-gate block (PyTorch row order i,f,g,o)."""
    W = np.array(W, np.float32, copy=True)
    W[2 * H:3 * H] *= f
    return W


def _reorder_w(W: np.ndarray) -> np.ndarray:
    """[2048, Kin] (rows i,f,g,o) -> [128, KC*2048] bf16 streaming layout.

    free index = kc*2048 + jc*512 + gt*128 + hp, partition = k (h within
    contraction chunk kc)."""
    Kin = W.shape[1]
    KC = Kin // 128
    Wg = W.reshape(4, 4, 128, KC, 128)[GATE_PERM]  # [gt, jc, hp, kc, k]
    return np.ascontiguousarray(
        Wg.transpose(4, 3, 1, 0, 2).reshape(128, KC * 2048)
    ).astype(ml_dtypes.bfloat16)


def _reorder_w_small(Waug: np.ndarray) -> np.ndarray:
    """[2048, Kin<=128] -> [Kin, 2048] bf16; free = jc*512 + gt*128 + hp."""
    Kin = Waug.shape[1]
    Wg = Waug.reshape(4, 4, 128, Kin)[GATE_PERM]  # [gt, jc, hp, k]
    return np.ascontiguousarray(
        Wg.transpose(3, 1, 0, 2).reshape(Kin, 2048)
    ).astype(ml_dtypes.bfloat16)


def _reorder_b(b: np.ndarray) -> np.ndarray:
    bg = b.reshape(4, 4, 128)[GATE_PERM]  # [gt, jc, hp]
    return np.ascontiguousarray(
        bg.transpose(1, 0, 2).reshape(1, 2048)
    ).astype(ml_dtypes.bfloat16)


def build_nc(thr1: float, thr2: float, t_steps: int):
    import concourse.bacc as bacc
    import concourse.mybir as mybir
    from concourse import tile, masks
    from concourse.tile import add_dep_helper

    f32 = mybir.dt.float32
    bf16 = mybir.dt.bfloat16
    AF = mybir.ActivationFunctionType
    OP = mybir.AluOpType

    nc = bacc.Bacc("TRN2", target_bir_lowering=False, debug=False,
                   num_devices=N_CORES)

    d_x = nc.dram_tensor("xin", [15, t_steps * BL], bf16, kind="ExternalInput")
    d_wih1 = nc.dram_tensor("wih1", [15, 2048], bf16, kind="ExternalInput")
    d_whh1 = nc.dram_tensor("whh1", [128, 4 * 2048], bf16,
                            kind="ExternalInput")
    d_w2 = nc.dram_tensor("w2", [128, 8 * 2048], bf16, kind="ExternalInput")
    d_b2 = nc.dram_tensor("b2r", [1, 2048], bf16, kind="ExternalInput")
    d_out = nc.dram_tensor("msum", [128, 128], f32, kind="ExternalOutput")

    with tile.TileContext(nc) as tc:
        with (
            tc.tile_pool(name="const", bufs=1) as cpool,
            tc.tile_pool(name="state", bufs=1) as spool,
            tc.tile_pool(name="gs", bufs=2) as gspool,
            tc.tile_pool(name="tmp", bufs=2) as tpool,
            tc.tile_pool(name="tsb", bufs=2) as tsbpool,
            tc.tile_pool(name="g1", bufs=2, space="PSUM") as g1pool,
            tc.tile_pool(name="g2", bufs=3, space="PSUM") as g2pool,
            tc.tile_pool(name="tp", bufs=2, space="PSUM") as tppool,
            tc.tile_pool(name="jk", bufs=1, space="PSUM") as jkpool,
        ):
            x_sb = cpool.tile([15, t_steps * BL], bf16, tag="x")
            wih1 = cpool.tile([15, 2048], bf16, tag="wih1")
            whh1 = cpool.tile([128, 4 * 2048], bf16, tag="whh1")
            w2 = cpool.tile([128, 8 * 2048], bf16, tag="w2")
            b2r = cpool.tile([1, 2048], bf16, tag="b2r")
            ident = cpool.tile([128, 128], bf16, tag="ident")
            ones = cpool.tile([1, BL], bf16, tag="ones")

            nc.sync.dma_start(x_sb[:], d_x[:])
            nc.sync.dma_start(wih1[:], d_wih1[:])
            nc.sync.dma_start(whh1[:], d_whh1[:])
            nc.sync.dma_start(w2[:], d_w2[:])
            nc.sync.dma_start(b2r[:], d_b2[:])
            masks.make_identity(nc, ident[:])
            nc.gpsimd.memset(ones[:], 1.0)

            syn1 = spool.tile([128, 128], f32, tag="syn1")
            syn2 = spool.tile([128, 128], f32, tag="syn2")
            msum = spool.tile([128, 128], f32, tag="msum")
            m1T = spool.tile([128, 128], bf16, tag="m1T0")
            m2T = spool.tile([128, 128], bf16, tag="m2T0")
            sp1 = spool.tile([128, 128], bf16, tag="sp10")
            sp2 = spool.tile([128, 128], bf16, tag="sp20")
            for s in (syn1, syn2, msum):
                nc.vector.memset(s[:], 0.0)
            for s in (m1T, m2T, sp1, sp2):
                nc.gpsimd.memset(s[:], 0.0)

            def mm(psum, lhs, rhs, jc, start, stop):
                return nc.tensor.matmul(
                    psum[32 * jc:32 * jc + 32, :], lhs, rhs,
                    start=start, stop=stop, tile_position=(0, 32 * jc),
                    skip_group_check=True)

            def x_round(g1t, t):
                xsl = x_sb[:, t * BL:(t + 1) * BL]
                return [mm(g1t, xsl, wih1[:, 512 * jc:512 * jc + 512], jc,
                           True, False) for jc in range(4)]

            def bias_round(g2t):
                return [mm(g2t, ones[0:1, :],
                           b2r[0:1, 512 * jc:512 * jc + 512], jc,
                           True, False) for jc in range(4)]

            def whh1_rounds(g1t, lhsT):
                for kc in range(4):
                    lhs = lhsT[:, 32 * kc:32 * kc + 32]
                    for jc in range(4):
                        off = 2048 * kc + 512 * jc
                        mm(g1t, lhs, whh1[:, off:off + 512], jc,
                           False, kc == 3)

            def wih2_rounds(g2t, lhsT):
                for kc in range(4):  # w2 chunks 0..3 = W_ih2
                    lhs = lhsT[:, 32 * kc:32 * kc + 32]
                    for jc in range(4):
                        off = 2048 * kc + 512 * jc
                        mm(g2t, lhs, w2[:, off:off + 512], jc, False, False)

            def whh2_rounds(g2t, lhsT):
                for kc in range(4):  # w2 chunks 4..7 = W_hh2, closes group
                    lhs = lhsT[:, 32 * kc:32 * kc + 32]
                    for jc in range(4):
                        off = 2048 * (4 + kc) + 512 * jc
                        mm(g2t, lhs, w2[:, off:off + 512], jc,
                           False, kc == 3)

            junk = jkpool.tile([128, 512], f32, tag="junk")

            def junk_mm(anchor=None):
                j = nc.tensor.matmul(
                    junk[0:32, :], ones[0:1, :], b2r[0:1, 0:512],
                    start=True, stop=True, tile_position=(0, 0),
                    skip_group_check=True)
                if anchor is not None:
                    add_dep_helper(j.ins, anchor.ins, sync=True,
                                   reason="ham keepalive")
                return j

            # ---- prologue ----
            # HAM warm-up: ~4us of junk matmuls so the PE clock ungates
            for _ in range(12):
                junk_mm()

            g1_cur = g1pool.tile([128, 512], f32, tag="g1")
            x_round(g1_cur, 0)
            whh1_rounds(g1_cur, m1T)  # m1T == 0: closes g1(0) uniformly
            g2t0 = g2pool.tile([128, 512], f32, tag="g2")
            g2_open = {0: g2t0}
            bias_round(g2t0)

            l2 = None  # staged layer-2 work: (g2 tile, sp1 tile, m2T_prev)

            for k in range(t_steps + 1):
                do1 = k < t_steps  # layer-1 step k exists
                s = k - 1          # layer-2 step handled this iteration

                # 1. close g2(s): wih2 @ spk1(s) + whh2 @ m2T(s-1)
                if l2 is not None:
                    g2s, sp1s, m2Tprev = l2
                    wih2_rounds(g2s, sp1s)
                    whh2_rounds(g2s, m2Tprev)

                # 2. open g1(k+1) / g2(k+1)
                g1_next = None
                if do1 and k + 1 < t_steps:
                    g1_next = g1pool.tile([128, 512], f32, tag="g1")
                    x_round(g1_next, k + 1)
                if do1 and k + 1 <= t_steps - 1:
                    g2n = g2pool.tile([128, 512], f32, tag="g2")
                    bias_round(g2n)
                    g2_open[k + 1] = g2n

                # 3. layer-1 elementwise chain + transpose + whh1(k+1)
                if do1:
                    gs1 = gspool.tile([128, 512], f32, tag="gs1")
                    nc.scalar.activation(gs1[:], g1_cur[:], AF.Sigmoid)
                    si, sf = gs1[:, 0:128], gs1[:, 128:256]
                    so, sg2 = gs1[:, 256:384], gs1[:, 384:512]
                    p1 = tpool.tile([128, 128], bf16, tag="p1")
                    i_p1 = nc.vector.scalar_tensor_tensor(
                        p1[:], sg2, 0.5, si, OP.subtract, OP.mult)
                    # p2 on DVE right after p1: gpsimd picked it up ~1us
                    # late (FIFO quirks), stalling syn1 on the L1 cycle
                    p2 = tpool.tile([128, 128], bf16, tag="p2")
                    i_p2 = nc.vector.tensor_tensor(p2[:], sf, syn1[:],
                                                   OP.mult)
                    i_sy1 = nc.vector.scalar_tensor_tensor(
                        syn1[:], p1[:], 2.0, p2[:], OP.mult, OP.add)
                    tc1 = tpool.tile([128, 128], f32, tag="tc1")
                    i_tc1 = nc.scalar.activation(tc1[:], syn1[:], AF.Tanh)
                    ht1 = tsbpool.tile([128, 128], bf16, tag="ht1")
                    nc.vector.tensor_tensor(ht1[:], so, tc1[:], OP.mult)

                    tp1 = tppool.tile([128, 128], bf16, tag="tp")
                    nc.tensor.transpose(tp1[:], ht1[:], ident[:])
                    m1T_new = tsbpool.tile([128, 128], bf16, tag="m1T")
                    sp1_new = tsbpool.tile([128, 128], bf16, tag="sp1")
                    with tc.high_priority():
                        nc.vector.scalar_tensor_tensor(
                            m1T_new[:], sp1[:], -thr1, tp1[:],
                            OP.mult, OP.add)
                        i_sp1 = nc.vector.tensor_scalar(
                            sp1_new[:], m1T_new[:], thr1, None, OP.is_gt)

                    if g1_next is not None:
                        whh1_rounds(g1_next, m1T_new)
                else:
                    i_sp1 = i_p2 = None

                # 4. layer-2 elementwise chain for step s (lagged)
                if s >= 0:
                    g2s = l2[0]
                    gs2 = gspool.tile([128, 512], f32, tag="gs2")
                    nc.scalar.activation(gs2[:], g2s[:], AF.Sigmoid)
                    si2, sf2 = gs2[:, 0:128], gs2[:, 128:256]
                    so2, sg22 = gs2[:, 256:384], gs2[:, 384:512]
                    q2 = tpool.tile([128, 128], f32, tag="q2")
                    i_q2 = nc.gpsimd.tensor_tensor(q2[:], sf2, syn2[:],
                                                   OP.mult)
                    q1 = tpool.tile([128, 128], f32, tag="q1")
                    i_q1 = nc.vector.scalar_tensor_tensor(
                        q1[:], sg22, 0.5, si2, OP.subtract, OP.mult)
                    if i_sp1 is not None:
                        # order-only edge (no HW semaphore): keep layer-1's
                        # critical m1T/sp1 ahead of q1 in the DVE FIFO
                        add_dep_helper(i_q1.ins, i_sp1.ins, sync=False,
                                       reason="dve fifo order")
                    i_sy2 = nc.vector.scalar_tensor_tensor(
                        syn2[:], q1[:], 2.0, q2[:], OP.mult, OP.add)
                    tc2t = tpool.tile([128, 128], f32, tag="tc2")
                    i_tc2 = nc.scalar.activation(tc2t[:], syn2[:], AF.Tanh)
                    ht2 = tsbpool.tile([128, 128], bf16, tag="ht2")
                    nc.vector.tensor_tensor(ht2[:], so2, tc2t[:], OP.mult)

                    tp2 = tppool.tile([128, 128], bf16, tag="tp")
                    nc.tensor.transpose(tp2[:], ht2[:], ident[:])
                    m2T_new = tsbpool.tile([128, 128], bf16, tag="m2T")
                    i_m2T = nc.vector.scalar_tensor_tensor(
                        m2T_new[:], sp2[:], -thr2, tp2[:], OP.mult, OP.add)
                    del i_m2T
                    sp2_new = tsbpool.tile([128, 128], bf16, tag="sp2")
                    nc.vector.tensor_scalar(sp2_new[:], m2T_new[:], thr2,
                                            None, OP.is_gt)
                    nc.gpsimd.tensor_tensor(msum[:], msum[:], m2T_new[:],
                                            OP.add)
                    m2T, sp2 = m2T_new, sp2_new

                # 5. stage next iteration's layer-2 work
                if do1:
                    l2 = (g2_open.pop(k), sp1_new, m2T)
                    m1T, sp1 = m1T_new, sp1_new
                    if g1_next is not None:
                        g1_cur = g1_next
                else:
                    l2 = None

            nc.sync.dma_start(d_out[:], msum[:])

    nc.compile()
    return nc


def prep_core_inputs(x, W_ih1, W_hh1, b_ih1, b_hh1, W_ih2, W_hh2,
                     b_ih2, b_hh2, t_steps):
    """Shared (weight) arrays + per-core x shards.  The g-gate rows are
    pre-scaled x2 so the kernel can use tanh(g) = 2*sigmoid(2g) - 1."""
    b1 = _scale_g(np.asarray(b_ih1, np.float32)
                  + np.asarray(b_hh1, np.float32), 2.0)
    wih1_aug = np.concatenate(
        [_scale_g(np.asarray(W_ih1, np.float32), 2.0), b1[:, None]],
        axis=1)  # [2048, 15]
    wih1_r = _reorder_w_small(wih1_aug)  # [15, 2048]
    whh1_r = _reorder_w(_scale_g(np.asarray(W_hh1, np.float32), 2.0))
    w2cat = np.concatenate(
        [_scale_g(np.asarray(W_ih2, np.float32), 2.0),
         _scale_g(np.asarray(W_hh2, np.float32), 2.0)],
        axis=1)  # [2048, 1024]
    w2_r = _reorder_w(w2cat)  # [128, 16384]
    b2_r = _reorder_b(_scale_g(np.asarray(b_ih2, np.float32)
                               + np.asarray(b_hh2, np.float32), 2.0))

    x = np.asarray(x, np.float32)[:t_steps]
    in_maps = []
    for c in range(N_CORES):
        xs = x[:, c * BL:(c + 1) * BL, :]  # [T, 32, 14]
        xt = np.ascontiguousarray(
            xs.transpose(2, 0, 1).reshape(I, t_steps * BL))
        x_aug = np.concatenate(
            [xt, np.ones((1, t_steps * BL), np.float32)],
            axis=0).astype(ml_dtypes.bfloat16)  # [15, T*32]
        in_maps.append({
            "xin": x_aug,
            "wih1": wih1_r,
            "whh1": whh1_r,
            "w2": w2_r,
            "b2r": b2_r,
        })
    return in_maps


def unpack_msum(msum: np.ndarray, t_steps: int) -> np.ndarray:
    """[128, 128] transposed accumulator [hp, 32*jc+b] -> [32, 512]."""
    return (msum.reshape(128, 4, 32).transpose(2, 1, 0).reshape(32, 512)
            / np.float32(t_steps))


def kernel(x, W_ih1, W_hh1, b_ih1, b_hh1, thr1,
           W_ih2, W_hh2, b_ih2, b_hh2, thr2, W_fc, b_fc):
    from concourse.bass_utils import run_bass_kernel_spmd

    t_steps = x.shape[0]
    key = (float(thr1), float(thr2), t_steps)
    if key not in _cache:
        _cache[key] = build_nc(float(thr1), float(thr2), t_steps)
    nc = _cache[key]

    in_maps = prep_core_inputs(x, W_ih1, W_hh1, b_ih1, b_hh1,
                               W_ih2, W_hh2, b_ih2, b_hh2, t_steps)
    res = run_bass_kernel_spmd(nc, in_maps, list(range(N_CORES)))

    W_fc = np.asarray(W_fc, np.float32)
    b_fc = np.asarray(b_fc, np.float32)
    out = np.empty((B, C), np.float32)
    for c in range(N_CORES):
        mean_c = unpack_msum(res.results[c]["msum"], t_steps)  # [32, 512]
        out[c * BL:(c + 1) * BL] = mean_c @ W_fc.T + b_fc
    return out


# revision 31
# speedup vs baseline: 1.1035x; 1.0991x over previous
"""Trainium2 Bass kernel for a 2-layer spiking LSTM (SLSTM) + FC readout.

Contract: kernel(**inputs) takes the FULL unsharded inputs and returns
the FULL [256, 8] output. Internally the batch is sharded 32-per-core
across 8 NeuronCores (data parallel, weights replicated); the T=400
time scan runs fully SBUF-resident per core.

v2 design (software-pipelined, layer 2 lags layer 1 by one iteration):

  - Gates for one layer in one PSUM bank [128, 512] f32:
      partition p = 32*jc + b, free n = 128*gt + hp, gt in [i, f, o, g].
    4-way PE column tiling streams the (bf16) weights; the state slice
    [128, 32] is stationary.
  - One 512-wide Sigmoid ACT per layer: the g-gate weight rows are
    pre-scaled x2 on the host so tanh(g) = 2*sigmoid(2g) - 1; the
    2s-1 folds into fused scalar_tensor_tensor DVE ops:
      p1 = (sg2 - 0.5) * si ;  syn' = 2*p1 + (sf * syn)
  - The PE transposes ht = so*tanh(syn') (pre-reset); the reset
    subtraction happens post-transpose, fused into the PSUM->SBUF
    cast:  mT = htT - thr*spkT_prev  (spike == reset indicator).
  - spkT = (mT > thr).  No separate reset tile, no scalar copy.
  - Layer 2 is emitted one iteration behind layer 1; its elementwise
    chain overlaps layer-1's matmuls.  A sync=False (order-only)
    dependency keeps layer-1's critical m1T/sp1 ahead of layer-2's q1
    in the DVE FIFO — this is worth ~850ns/step; sync=True semaphore
    edges or paced keepalive matmuls both destabilize the schedule and
    make the HAM clock-gate oscillation worse (measured).
    PSUM pools: g1 x2, g2 x3, tp x2, junk x1 banks.
"""

import sys

sys.path.insert(0, "/opt/trn_rl_repo")

import numpy as np
import ml_dtypes

T, B, I, H, C = 400, 256, 14, 512, 8
N_CORES = 8
BL = B // N_CORES  # 32
GATE_PERM = [1, 0, 3, 2]  # PyTorch gate rows [i,f,g,o] -> our order [f,i,o,g]

_cache = {}


def _scale_g(W: np.ndarray, f: float) -> np.ndarray:
    """Scale the g-gate block (PyTorch row order i,f,g,o)."""
    W = np.array(W, np.float32, copy=True)
    W[2 * H:3 * H] *= f
    return W


def _reorder_w(W: np.ndarray) -> np.ndarray:
    """[2048, Kin] (rows i,f,g,o) -> [128, KC*2048] bf16 streaming layout.

    free index = kc*2048 + jc*512 + gt*128 + hp, partition = k (h within
    contraction chunk kc)."""
    Kin = W.shape[1]
    KC = Kin // 128
    Wg = W.reshape(4, 4, 128, KC, 128)[GATE_PERM]  # [gt, jc, hp, kc, k]
    return np.ascontiguousarray(
        Wg.transpose(4, 3, 1, 0, 2).reshape(128, KC * 2048)
    ).astype(ml_dtypes.bfloat16)


def _reorder_w_small(Waug: np.ndarray) -> np.ndarray:
    """[2048, Kin<=128] -> [Kin, 2048] bf16; free = jc*512 + gt*128 + hp."""
    Kin = Waug.shape[1]
    Wg = Waug.reshape(4, 4, 128, Kin)[GATE_PERM]  # [gt, jc, hp, k]
    return np.ascontiguousarray(
        Wg.transpose(3, 1, 0, 2).reshape(Kin, 2048)
    ).astype(ml_dtypes.bfloat16)


def _reorder_b(b: np.ndarray) -> np.ndarray:
    bg = b.reshape(4, 4, 128)[GATE_PERM]  # [gt, jc, hp]
    return np.ascontiguousarray(
        bg.transpose(1, 0, 2).reshape(1, 2048)
    ).astype(ml_dtypes.bfloat16)


def build_nc(thr1: float, thr2: float, t_steps: int):
    import concourse.bacc as bacc
    import concourse.mybir as mybir
    from concourse import tile, masks
    from concourse.tile import add_dep_helper

    f32 = mybir.dt.float32
    bf16 = mybir.dt.bfloat16
    AF = mybir.ActivationFunctionType
    OP = mybir.AluOpType

    nc = bacc.Bacc("TRN2", target_bir_lowering=False, debug=False,
                   num_devices=N_CORES)

    d_x = nc.dram_tensor("xin", [15, t_steps * BL], bf16, kind="ExternalInput")
    d_wih1 = nc.dram_tensor("wih1", [15, 2048], bf16, kind="ExternalInput")
    d_whh1 = nc.dram_tensor("whh1", [128, 4 * 2048], bf16,
                            kind="ExternalInput")
    d_w2 = nc.dram_tensor("w2", [128, 8 * 2048], bf16, kind="ExternalInput")
    d_b2 = nc.dram_tensor("b2r", [1, 2048], bf16, kind="ExternalInput")
    d_out = nc.dram_tensor("msum", [128, 128], f32, kind="ExternalOutput")

    with tile.TileContext(nc) as tc:
        with (
            tc.tile_pool(name="const", bufs=1) as cpool,
            tc.tile_pool(name="state", bufs=1) as spool,
            tc.tile_pool(name="gs", bufs=2) as gspool,
            tc.tile_pool(name="tmp", bufs=2) as tpool,
            tc.tile_pool(name="tsb", bufs=2) as tsbpool,
            tc.tile_pool(name="g1", bufs=2, space="PSUM") as g1pool,
            tc.tile_pool(name="g2", bufs=3, space="PSUM") as g2pool,
            tc.tile_pool(name="tp", bufs=2, space="PSUM") as tppool,
            tc.tile_pool(name="jk", bufs=1, space="PSUM") as jkpool,
        ):
            x_sb = cpool.tile([15, t_steps * BL], bf16, tag="x")
            wih1 = cpool.tile([15, 2048], bf16, tag="wih1")
            whh1 = cpool.tile([128, 4 * 2048], bf16, tag="whh1")
            w2 = cpool.tile([128, 8 * 2048], bf16, tag="w2")
            b2r = cpool.tile([1, 2048], bf16, tag="b2r")
            ident = cpool.tile([128, 128], bf16, tag="ident")
            ones = cpool.tile([1, BL], bf16, tag="ones")

            nc.sync.dma_start(x_sb[:], d_x[:])
            nc.sync.dma_start(wih1[:], d_wih1[:])
            nc.sync.dma_start(whh1[:], d_whh1[:])
            nc.sync.dma_start(w2[:], d_w2[:])
            nc.sync.dma_start(b2r[:], d_b2[:])
            masks.make_identity(nc, ident[:])
            nc.gpsimd.memset(ones[:], 1.0)

            syn1 = spool.tile([128, 128], f32, tag="syn1")
            syn2 = spool.tile([128, 128], f32, tag="syn2")
            msum = spool.tile([128, 128], f32, tag="msum")
            m1T = spool.tile([128, 128], bf16, tag="m1T0")
            m2T = spool.tile([128, 128], bf16, tag="m2T0")
            sp1 = spool.tile([128, 128], bf16, tag="sp10")
            sp2 = spool.tile([128, 128], bf16, tag="sp20")
            for s in (syn1, syn2, msum):
                nc.vector.memset(s[:], 0.0)
            for s in (m1T, m2T, sp1, sp2):
                nc.gpsimd.memset(s[:], 0.0)

            def mm(psum, lhs, rhs, jc, start, stop):
                return nc.tensor.matmul(
                    psum[32 * jc:32 * jc + 32, :], lhs, rhs,
                    start=start, stop=stop, tile_position=(0, 32 * jc),
                    skip_group_check=True)

            def x_round(g1t, t):
                xsl = x_sb[:, t * BL:(t + 1) * BL]
                return [mm(g1t, xsl, wih1[:, 512 * jc:512 * jc + 512], jc,
                           True, False) for jc in range(4)]

            def bias_round(g2t):
                return [mm(g2t, ones[0:1, :],
                           b2r[0:1, 512 * jc:512 * jc + 512], jc,
                           True, False) for jc in range(4)]

            def whh1_rounds(g1t, lhsT):
                for kc in range(4):
                    lhs = lhsT[:, 32 * kc:32 * kc + 32]
                    for jc in range(4):
                        off = 2048 * kc + 512 * jc
                        mm(g1t, lhs, whh1[:, off:off + 512], jc,
                           False, kc == 3)

            def wih2_rounds(g2t, lhsT):
                for kc in range(4):  # w2 chunks 0..3 = W_ih2
                    lhs = lhsT[:, 32 * kc:32 * kc + 32]
                    for jc in range(4):
                        off = 2048 * kc + 512 * jc
                        mm(g2t, lhs, w2[:, off:off + 512], jc, False, False)

            def whh2_rounds(g2t, lhsT):
                for kc in range(4):  # w2 chunks 4..7 = W_hh2, closes group
                    lhs = lhsT[:, 32 * kc:32 * kc + 32]
                    for jc in range(4):
                        off = 2048 * (4 + kc) + 512 * jc
                        mm(g2t, lhs, w2[:, off:off + 512], jc,
                           False, kc == 3)

            junk = jkpool.tile([128, 512], f32, tag="junk")

            def junk_mm(anchor=None):
                j = nc.tensor.matmul(
                    junk[0:32, :], ones[0:1, :], b2r[0:1, 0:512],
                    start=True, stop=True, tile_position=(0, 0),
                    skip_group_check=True)
                if anchor is not None:
                    add_dep_helper(j.ins, anchor.ins, sync=True,
                                   reason="ham keepalive")
                return j

            # ---- prologue ----
            # HAM warm-up: ~4us of junk matmuls so the PE clock ungates
            for _ in range(12):
                junk_mm()

            g1_cur = g1pool.tile([128, 512], f32, tag="g1")
            x_round(g1_cur, 0)
            whh1_rounds(g1_cur, m1T)  # m1T == 0: closes g1(0) uniformly
            g2t0 = g2pool.tile([128, 512], f32, tag="g2")
            g2_open = {0: g2t0}
            bias_round(g2t0)

            l2 = None  # staged layer-2 work: (g2 tile, sp1 tile, m2T_prev)

            for k in range(t_steps + 1):
                do1 = k < t_steps  # layer-1 step k exists
                s = k - 1          # layer-2 step handled this iteration

                # 1. close g2(s): wih2 @ spk1(s) + whh2 @ m2T(s-1)
                if l2 is not None:
                    g2s, sp1s, m2Tprev = l2
                    wih2_rounds(g2s, sp1s)
                    whh2_rounds(g2s, m2Tprev)

                # 2. open g1(k+1) / g2(k+1)
                g1_next = None
                if do1 and k + 1 < t_steps:
                    g1_next = g1pool.tile([128, 512], f32, tag="g1")
                    x_round(g1_next, k + 1)
                if do1 and k + 1 <= t_steps - 1:
                    g2n = g2pool.tile([128, 512], f32, tag="g2")
                    bias_round(g2n)
                    g2_open[k + 1] = g2n

                # 3. layer-1 elementwise chain + transpose + whh1(k+1)
                if do1:
                    gs1 = gspool.tile([128, 512], f32, tag="gs1")
                    nc.scalar.activation(gs1[:, 0:256], g1_cur[:, 0:256],
                                         AF.Sigmoid)
                    nc.scalar.activation(gs1[:, 256:512],
                                         g1_cur[:, 256:512], AF.Sigmoid)
                    sf, si = gs1[:, 0:128], gs1[:, 128:256]
                    so, sg2 = gs1[:, 256:384], gs1[:, 384:512]
                    p1 = tpool.tile([128, 128], bf16, tag="p1")
                    i_p1 = nc.vector.scalar_tensor_tensor(
                        p1[:], sg2, 0.5, si, OP.subtract, OP.mult)
                    # p2 on DVE right after p1: gpsimd picked it up ~1us
                    # late (FIFO quirks), stalling syn1 on the L1 cycle
                    p2 = tpool.tile([128, 128], bf16, tag="p2")
                    i_p2 = nc.vector.tensor_tensor(p2[:], sf, syn1[:],
                                                   OP.mult)
                    i_sy1 = nc.vector.scalar_tensor_tensor(
                        syn1[:], p1[:], 2.0, p2[:], OP.mult, OP.add)
                    tc1 = tpool.tile([128, 128], f32, tag="tc1")
                    i_tc1 = nc.scalar.activation(tc1[:], syn1[:], AF.Tanh)
                    ht1 = tsbpool.tile([128, 128], bf16, tag="ht1")
                    nc.vector.tensor_tensor(ht1[:], so, tc1[:], OP.mult)

                    tp1 = tppool.tile([128, 128], bf16, tag="tp")
                    nc.tensor.transpose(tp1[:], ht1[:], ident[:])
                    m1T_new = tsbpool.tile([128, 128], bf16, tag="m1T")
                    sp1_new = tsbpool.tile([128, 128], bf16, tag="sp1")
                    with tc.high_priority():
                        nc.vector.scalar_tensor_tensor(
                            m1T_new[:], sp1[:], -thr1, tp1[:],
                            OP.mult, OP.add)
                        i_sp1 = nc.vector.tensor_scalar(
                            sp1_new[:], m1T_new[:], thr1, None, OP.is_gt)

                    if g1_next is not None:
                        whh1_rounds(g1_next, m1T_new)
                else:
                    i_sp1 = i_p2 = None

                # 4. layer-2 elementwise chain for step s (lagged)
                if s >= 0:
                    g2s = l2[0]
                    gs2 = gspool.tile([128, 512], f32, tag="gs2")
                    nc.scalar.activation(gs2[:, 0:256], g2s[:, 0:256],
                                         AF.Sigmoid)
                    nc.scalar.activation(gs2[:, 256:512],
                                         g2s[:, 256:512], AF.Sigmoid)
                    sf2, si2 = gs2[:, 0:128], gs2[:, 128:256]
                    so2, sg22 = gs2[:, 256:384], gs2[:, 384:512]
                    q2 = tpool.tile([128, 128], f32, tag="q2")
                    i_q2 = nc.gpsimd.tensor_tensor(q2[:], sf2, syn2[:],
                                                   OP.mult)
                    q1 = tpool.tile([128, 128], f32, tag="q1")
                    i_q1 = nc.vector.scalar_tensor_tensor(
                        q1[:], sg22, 0.5, si2, OP.subtract, OP.mult)
                    if i_sp1 is not None:
                        # order-only edge (no HW semaphore): keep layer-1's
                        # critical m1T/sp1 ahead of q1 in the DVE FIFO
                        add_dep_helper(i_q1.ins, i_sp1.ins, sync=False,
                                       reason="dve fifo order")
                    i_sy2 = nc.vector.scalar_tensor_tensor(
                        syn2[:], q1[:], 2.0, q2[:], OP.mult, OP.add)
                    tc2t = tpool.tile([128, 128], f32, tag="tc2")
                    i_tc2 = nc.scalar.activation(tc2t[:], syn2[:], AF.Tanh)
                    ht2 = tsbpool.tile([128, 128], bf16, tag="ht2")
                    nc.vector.tensor_tensor(ht2[:], so2, tc2t[:], OP.mult)

                    tp2 = tppool.tile([128, 128], bf16, tag="tp")
                    nc.tensor.transpose(tp2[:], ht2[:], ident[:])
                    m2T_new = tsbpool.tile([128, 128], bf16, tag="m2T")
                    i_m2T = nc.vector.scalar_tensor_tensor(
                        m2T_new[:], sp2[:], -thr2, tp2[:], OP.mult, OP.add)
                    del i_m2T
                    sp2_new = tsbpool.tile([128, 128], bf16, tag="sp2")
                    nc.vector.tensor_scalar(sp2_new[:], m2T_new[:], thr2,
                                            None, OP.is_gt)
                    nc.gpsimd.tensor_tensor(msum[:], msum[:], m2T_new[:],
                                            OP.add)
                    m2T, sp2 = m2T_new, sp2_new

                # 5. stage next iteration's layer-2 work
                if do1:
                    l2 = (g2_open.pop(k), sp1_new, m2T)
                    m1T, sp1 = m1T_new, sp1_new
                    if g1_next is not None:
                        g1_cur = g1_next
                else:
                    l2 = None

            nc.sync.dma_start(d_out[:], msum[:])

    nc.compile()
    return nc


def prep_core_inputs(x, W_ih1, W_hh1, b_ih1, b_hh1, W_ih2, W_hh2,
                     b_ih2, b_hh2, t_steps):
    """Shared (weight) arrays + per-core x shards.  The g-gate rows are
    pre-scaled x2 so the kernel can use tanh(g) = 2*sigmoid(2g) - 1."""
    b1 = _scale_g(np.asarray(b_ih1, np.float32)
                  + np.asarray(b_hh1, np.float32), 2.0)
    wih1_aug = np.concatenate(
        [_scale_g(np.asarray(W_ih1, np.float32), 2.0), b1[:, None]],
        axis=1)  # [2048, 15]
    wih1_r = _reorder_w_small(wih1_aug)  # [15, 2048]
    whh1_r = _reorder_w(_scale_g(np.asarray(W_hh1, np.float32), 2.0))
    w2cat = np.concatenate(
        [_scale_g(np.asarray(W_ih2, np.float32), 2.0),
         _scale_g(np.asarray(W_hh2, np.float32), 2.0)],
        axis=1)  # [2048, 1024]
    w2_r = _reorder_w(w2cat)  # [128, 16384]
    b2_r = _reorder_b(_scale_g(np.asarray(b_ih2, np.float32)
                               + np.asarray(b_hh2, np.float32), 2.0))

    x = np.asarray(x, np.float32)[:t_steps]
    in_maps = []
    for c in range(N_CORES):
        xs = x[:, c * BL:(c + 1) * BL, :]  # [T, 32, 14]
        xt = np.ascontiguousarray(
            xs.transpose(2, 0, 1).reshape(I, t_steps * BL))
        x_aug = np.concatenate(
            [xt, np.ones((1, t_steps * BL), np.float32)],
            axis=0).astype(ml_dtypes.bfloat16)  # [15, T*32]
        in_maps.append({
            "xin": x_aug,
            "wih1": wih1_r,
            "whh1": whh1_r,
            "w2": w2_r,
            "b2r": b2_r,
        })
    return in_maps


def unpack_msum(msum: np.ndarray, t_steps: int) -> np.ndarray:
    """[128, 128] transposed accumulator [hp, 32*jc+b] -> [32, 512]."""
    return (msum.reshape(128, 4, 32).transpose(2, 1, 0).reshape(32, 512)
            / np.float32(t_steps))


def kernel(x, W_ih1, W_hh1, b_ih1, b_hh1, thr1,
           W_ih2, W_hh2, b_ih2, b_hh2, thr2, W_fc, b_fc):
    from concourse.bass_utils import run_bass_kernel_spmd

    t_steps = x.shape[0]
    key = (float(thr1), float(thr2), t_steps)
    if key not in _cache:
        _cache[key] = build_nc(float(thr1), float(thr2), t_steps)
    nc = _cache[key]

    in_maps = prep_core_inputs(x, W_ih1, W_hh1, b_ih1, b_hh1,
                               W_ih2, W_hh2, b_ih2, b_hh2, t_steps)
    res = run_bass_kernel_spmd(nc, in_maps, list(range(N_CORES)))

    W_fc = np.asarray(W_fc, np.float32)
    b_fc = np.asarray(b_fc, np.float32)
    out = np.empty((B, C), np.float32)
    for c in range(N_CORES):
        mean_c = unpack_msum(res.results[c]["msum"], t_steps)  # [32, 512]
        out[c * BL:(c + 1) * BL] = mean_c @ W_fc.T + b_fc
    return out
